# revision 19
# baseline (speedup 1.0000x reference)
"""Trainium2 Bass kernel for HPA-style attention (nn_Attention_33423435497672).

Reference computation (B=4, N=1024, C=1024, H=16, D=64):
    q  = xq @ Wq.T                      -> [B,N,C] -> heads [B,H,N,64]
    kv = xk @ Wkv.T ; k,v = split(kv)   -> [B,H,N,64] each  (xv unused)
    attn = (q @ k^T) * D**-0.5          -> [B,H,N,N]   (pre-softmax, saved)
    p = softmax(attn); x = p @ v        -> [B,N,C]
    out = x @ Wp.T + bp
    returns (out.transpose(1,0,2) [N,B,C], attn.sum(heads)/H [B,N,N])

Sharding: 8 cores = 4 batches x 2 query-halves.  Core c handles batch c//2,
query rows (c%2)*512..+512.  Each core computes the full K/V projection for
its batch (duplicated across the pair), so no collectives are needed; both
outputs partition cleanly by (batch, query-row) and the host reassembles.

On-device everything is kept transposed (contraction dim on partitions);
the host pre-transposes inputs and un-transposes outputs.  Matmuls run as
float32r (the full-rate fp32 PE path).  Softmax skips max-subtraction
(logits are ~N(0, 0.17), exp cannot overflow).

The per-head softmax denominator comes free from the AV matmul: v is stored
with 65 columns per head, the 65th column being all-ones, so row 64 of the
AV psum is sum_nk(exp).  The row is reciprocal'd and broadcast back across
partitions with a K=1 matmul against a ones row.
"""

import sys

sys.path.insert(0, "/opt/trn_rl_repo")

import numpy as np

import concourse.bass as bass
import concourse.mybir as mybir
from concourse import tile

B, N, C, H = 4, 1024, 1024, 16
D = C // H          # 64
NQ = N // 2         # 512 query rows per core
P = 128
F32 = mybir.dt.float32
BD = mybir.dt.bfloat16
SCALE = float(D) ** -0.5          # 0.125
ATTN_SCALE = SCALE / H            # 1/128

CT = C // P         # 8 c-tiles
NKT = N // P        # 8 nk-tiles
VW = 2 * D          # 128: 64 v columns + 64 ones columns per head
SCB = 2             # nk-chunks batched per scores psum tile

_CACHE = {}




def build_nc():
    nc = bass.Bass(target_bir_lowering=False)
    Exp = mybir.ActivationFunctionType.Exp

    xqT = nc.declare_dram_parameter("xqT", [C, NQ], BD, isOutput=False)
    xkT = nc.declare_dram_parameter("xkT", [C, N], BD, isOutput=False)
    wqT = nc.declare_dram_parameter("wqT", [C, C], BD, isOutput=False)
    wkvT = nc.declare_dram_parameter("wkvT", [C, 2 * C], BD, isOutput=False)
    wpT = nc.declare_dram_parameter("wpT", [C, C], BD, isOutput=False)
    bp = nc.declare_dram_parameter("bp", [C, 1], F32, isOutput=False)
    outT = nc.declare_dram_parameter("outT", [C, NQ], F32, isOutput=True)
    attnT = nc.declare_dram_parameter("attnT", [N, NQ], F32, isOutput=True)

    with nc.allow_low_precision(reason="bf16 compute path"), \
         tile.TileContext(nc) as tc:
        with (
            tc.tile_pool(name="consts", bufs=1) as consts,
            tc.tile_pool(name="acts", bufs=1) as acts,
            tc.tile_pool(name="wpool", bufs=1) as wpool,
            tc.tile_pool(name="stage", bufs=3) as stage,
            tc.tile_pool(name="scp", bufs=3, space="PSUM") as scp,
            tc.tile_pool(name="avp", bufs=2, space="PSUM") as avp,
        ):
            bias_sb = consts.tile([P, CT], F32, name="bias", tag="bias")
            nc.sync.dma_start(bias_sb[:], bp.rearrange("(t p) o -> p (t o)", p=P))
            bias_warm = consts.tile([P, CT], F32, name="bias_warm", tag="bias_warm")
            nc.vector.tensor_copy(bias_warm[:], bias_sb[:])

            # ---- input activations (pre-transposed on host) ----
            xq_sb = [acts.tile([P, NQ], BD, name=f"xq{i}", tag=f"xq{i}") for i in range(CT)]
            xk_sb = [acts.tile([P, N], BD, name=f"xk{i}", tag=f"xk{i}") for i in range(CT)]
            for i in range(CT):
                nc.sync.dma_start(xq_sb[i][:], xqT[i * P:(i + 1) * P, :])

            qT_sb = [acts.tile([P, NQ], BD, name=f"qT{i}", tag=f"qT{i}") for i in range(CT)]
            kT_sb = [acts.tile([P, N], BD, name=f"kT{i}", tag=f"kT{i}") for i in range(CT)]
            v_sb = [acts.tile([P, H, VW], BD, name=f"v{i}", tag=f"v{i}") for i in range(NKT)]
            xT_sb = [acts.tile([P, NQ], BD, name=f"xT{i}", tag=f"xT{i}") for i in range(CT)]

            # ---- PE warmup: dummy matmuls cover the initial DMA stall and
            # get the HAM activity window hot before real work arrives ----
            dum = consts.tile([1, NQ], BD, name="dum", tag="dum")
            nc.vector.memset(dum[:], 0.0)
            for _ in range(6):
                wps = avp.tile([VW, NQ], F32, name="av", tag="av")
                for r in range(8):
                    nc.tensor.matmul(wps[0:1, :], dum[:, 0:1], dum[:],
                                     start=(r == 0), stop=(r == 7),
                                     skip_group_check=True)

            # ---- fused emission: projection / attention / attn-sum groups
            # interleaved so the PE never idles long enough to re-throttle ----
            wq_t = [wpool.tile([P, C], BD, name=f"wq_t{i}", tag=f"wq{i}") for i in range(CT)]
            for ct in range(CT):
                nc.sync.dma_start(wq_t[ct][:], wqT[ct * P:(ct + 1) * P, :])
            wk_t = [wpool.tile([P, C], BD, name=f"wk_t{i}", tag=f"wk{i}") for i in range(CT)]
            for ct in range(CT):
                nc.sync.dma_start(wk_t[ct][:], wkvT[ct * P:(ct + 1) * P, 0:C])
            for i in range(CT):
                nc.sync.dma_start(xk_sb[i][:], xkT[i * P:(i + 1) * P, :])
            wv_t = [wpool.tile([P, C], BD, name=f"wv_t{i}", tag=f"wv{i}") for i in range(CT)]
            for ct in range(CT):
                nc.sync.dma_start(wv_t[ct][:], wkvT[ct * P:(ct + 1) * P, C:2 * C])

            def emit_A(ot):
                ps = scp.tile([P, NQ], F32, name="pp", tag="sc")
                for ct in range(CT):
                    nc.tensor.matmul(
                        ps[:], wq_t[ct][:, ot * P:(ot + 1) * P], xq_sb[ct][:],
                        start=(ct == 0), stop=(ct == CT - 1))
                nc.vector.tensor_copy(qT_sb[ot][:], ps[:])

            def emit_B1(ot, hf):
                ps = scp.tile([P, NQ], F32, name="pp", tag="sc")
                for ct in range(CT):
                    nc.tensor.matmul(
                        ps[:], wk_t[ct][:, ot * P:(ot + 1) * P],
                        xk_sb[ct][:, hf * NQ:(hf + 1) * NQ],
                        start=(ct == 0), stop=(ct == CT - 1))
                nc.vector.tensor_copy(kT_sb[ot][:, hf * NQ:(hf + 1) * NQ], ps[:])

            def emit_B2(nt, hf):
                ps = scp.tile([P, NQ], F32, name="pp", tag="sc")
                for ct in range(CT):
                    nc.tensor.matmul(
                        ps[:], xk_sb[ct][:, nt * P:(nt + 1) * P],
                        wv_t[ct][:, hf * NQ:(hf + 1) * NQ],
                        start=(ct == 0), stop=(ct == CT - 1))
                nc.vector.tensor_copy(
                    v_sb[nt][:, hf * (H // 2):(hf + 1) * (H // 2), 0:D],
                    ps.rearrange("p (h d) -> p h d", d=D))

            def emit_E(nt):
                ps = scp.tile([P, NQ], F32, name="pp", tag="sc")
                for ct in range(CT):
                    nc.tensor.matmul(
                        ps[:], kT_sb[ct][:, nt * P:(nt + 1) * P], qT_sb[ct][:],
                        start=(ct == 0), stop=(ct == CT - 1))
                ast = stage.tile([P, NQ], F32, name=f"ast{nt}", tag=f"ast{nt}", bufs=1)
                nc.vector.tensor_scalar_mul(ast[:], ps[:], ATTN_SCALE)
                nc.sync.dma_start(attnT[nt * P:(nt + 1) * P, :], ast[:])

            def emit_head(h):
                ht, hr = h // 2, (h % 2) * D
                av = avp.tile([VW, NQ], F32, name="av", tag="av")
                for j in range(NKT // SCB):
                    sc = scp.tile([P, SCB, NQ], F32, name="sc", tag="sc")
                    for tt in range(SCB):
                        t = j * SCB + tt
                        nc.tensor.matmul(
                            sc[:, tt, :],
                            kT_sb[ht][hr:hr + D, t * P:(t + 1) * P],
                            qT_sb[ht][hr:hr + D, :],
                            start=True, stop=True)
                    ex = acts.tile([P, SCB, NQ], BD, name=f"ex{j}", tag=f"ex{j}")
                    nc.scalar.activation(ex[:], sc[:], Exp, scale=SCALE)
                    for tt in range(SCB):
                        t = j * SCB + tt
                        nc.tensor.matmul(
                            av[:], v_sb[t][:, h, :], ex[:, tt, :],
                            start=(t == 0), stop=(t == NKT - 1),
                            skip_group_check=True)
                rec = stage.tile([D, NQ], F32, name="rec", tag="rec")
                nc.vector.reciprocal(rec[:, 0:NQ // 2], av[D:VW, 0:NQ // 2])
                nc.vector.reciprocal(rec[:, NQ // 2:], av[D:VW, NQ // 2:])
                nc.vector.tensor_mul(xT_sb[ht][hr:hr + D, :], av[0:D, :], rec[:])

            # prelude: everything head 0 needs, plus v ones columns
            for nt in range(NKT):
                nc.vector.memset(v_sb[nt][:, :, D:VW], 1.0)
            emit_A(0)
            emit_B1(0, 0)
            emit_B1(0, 1)
            for nt in range(NKT):
                emit_B2(nt, 0)

            fillers = {
                0: [lambda: emit_A(1), lambda: emit_B1(1, 0), lambda: emit_B1(1, 1),
                    lambda: emit_B2(0, 1), lambda: emit_B2(1, 1)],
                1: [lambda: emit_A(2), lambda: emit_B2(2, 1), lambda: emit_B2(3, 1)],
                2: [lambda: emit_B1(2, 0), lambda: emit_B1(2, 1), lambda: emit_B2(4, 1)],
                3: [lambda: emit_A(3), lambda: emit_B2(5, 1), lambda: emit_B2(6, 1)],
                4: [lambda: emit_B1(3, 0), lambda: emit_B1(3, 1), lambda: emit_B2(7, 1)],
                5: [lambda: emit_A(4), lambda: emit_B1(4, 0), lambda: emit_B1(4, 1)],
                6: [lambda: emit_A(5), lambda: emit_B1(5, 0)],
                7: [lambda: emit_B1(5, 1), lambda: emit_A(6)],
                8: [lambda: emit_B1(6, 0), lambda: emit_B1(6, 1)],
                9: [lambda: emit_A(7), lambda: emit_B1(7, 0)],
                10: [lambda: emit_B1(7, 1)],
                12: [lambda: emit_E(0), lambda: emit_E(1)],
                13: [lambda: emit_E(2), lambda: emit_E(3)],
                14: [lambda: emit_E(4), lambda: emit_E(5)],
                15: [lambda: emit_E(6), lambda: emit_E(7)],
            }
            for h in range(H):
                emit_head(h)
                for f in fillers.get(h, []):
                    f()

            # ---- stage D: outT[co, nq] = sum_ci wpT[ci, co] xT[ci, nq] + bp ----
            wp_t = [wpool.tile([P, C], BD, name=f"wp_t{i}", tag=f"wq{i}") for i in range(CT)]
            for ct in range(CT):
                nc.sync.dma_start(wp_t[ct][:], wpT[ct * P:(ct + 1) * P, :])
            for ot in range(CT):
                ps = scp.tile([P, NQ], F32, name="pp", tag="sc")
                for ct in range(CT):
                    nc.tensor.matmul(
                        ps[:], wp_t[ct][:, ot * P:(ot + 1) * P], xT_sb[ct][:],
                        start=(ct == 0), stop=(ct == CT - 1))
                ost = stage.tile([P, NQ], F32, name=f"ost{ot}", tag=f"ost{ot}", bufs=1)
                nc.vector.tensor_scalar_add(ost[:], ps[:], bias_sb[:, ot:ot + 1])
                nc.sync.dma_start(outT[ot * P:(ot + 1) * P, :], ost[:])

    _fix_wait_overflow(nc)
    nc.finalize()
    return nc


def _fix_wait_overflow(nc):
    """Walrus's per-instruction ISA structs carry a single sync-wait slot,
    but Tile sometimes attaches two or three.  Three sound repairs:
    - DMA instructions: drop waits on a queue semaphore the instruction
      itself increments (per-engine descriptor FIFO makes them implicit);
    - compute engines: drop own-semaphore waits whose producing instruction
      retired >=3 instructions earlier on the same in-order engine;
    - matmuls: move leftover extra waits onto the immediately-preceding
      Ldweights (no waits, no updates, so no cycle risk)."""
    skip = ("InstDrain", "InstEventSemaphore")
    # Split over-subscribed tail drains: one wait per InstDrain.
    for block in nc.m.functions[0].blocks:
        edits = []
        for idx, inst in enumerate(block.instructions):
            si = getattr(inst, "sync_info", None)
            if (inst.__class__.__name__ == "InstDrain" and si is not None
                    and len(si.on_wait) > 1):
                extra = []
                while len(si.on_wait) > 1:
                    extra.append(si.on_wait.pop())
                pres = []
                for w in extra:
                    d = mybir.InstDrain(
                        name=nc.get_next_instruction_name(),
                        ins=[], outs=[], bass_is_fusable=False)
                    d.engine = inst.engine
                    d.sync_info = mybir.SyncInfo(on_wait=[w], on_update=[])
                    pres.append(d)
                edits.append((idx, pres))
        for idx, pres in reversed(edits):
            for d in reversed(pres):
                block.instructions.insert(idx, d)
    for block in nc.m.functions[0].blocks:
        pos_by_eng = {}
        prev_by_eng = {}
        inc_hist = {}      # (eng, sem) -> [(stream_pos, cum_after)]
        for inst in block.instructions:
            eng = str(getattr(inst, "engine", None))
            pos = pos_by_eng.get(eng, 0)
            si = getattr(inst, "sync_info", None)
            cls = inst.__class__.__name__
            if si is not None and len(si.on_wait) > 1 and cls not in skip:
                ups = {u.ant_name for u in si.on_update}
                keep = []
                for w in si.on_wait:
                    nm = getattr(w, "ant_name", "") or ""
                    if nm in ups and w.wait_value is not None:
                        if cls == "InstDMACopy":
                            continue                      # FIFO-implied
                        hist = inc_hist.get((eng, nm), [])
                        idx = next((p for p, cum in hist
                                    if cum >= w.wait_value), None)
                        if idx is not None and pos - idx - 1 >= 3:
                            continue                      # long retired
                    keep.append(w)
                while len(si.on_wait) > 0:
                    si.on_wait.pop()
                for w in keep:
                    si.on_wait.append(w)
                if len(si.on_wait) > 1:
                    prev = prev_by_eng.get(eng)
                    psi = prev.sync_info if prev is not None else None
                    if psi is None and prev is not None:
                        psi = mybir.SyncInfo(on_wait=[], on_update=[])
                        prev.sync_info = psi
                    if (psi is not None and len(psi.on_wait) == 0
                            and len(psi.on_update) == 0):
                        while len(si.on_wait) > 1:
                            psi.on_wait.append(si.on_wait.pop())
                assert len(si.on_wait) <= 1, (
                    f"{inst.name} ({cls}): still "
                    f"{[(w.ant_name, w.wait_value) for w in si.on_wait]}")
            if si is not None:
                for u in si.on_update:
                    key = (eng, u.ant_name)
                    hist = inc_hist.setdefault(key, [])
                    cum = hist[-1][1] if hist else 0
                    hist.append((pos, cum + (u.update_value or 0)))
            prev_by_eng[eng] = inst
            pos_by_eng[eng] = pos + 1


def make_in_maps(xq, xk, Wq, Wkv, Wp, bp):
    bf16 = mybir.dt.np(BD)
    wqT = np.ascontiguousarray(Wq.T).astype(bf16)
    wkvT = np.ascontiguousarray(Wkv.T).astype(bf16)
    wpT = np.ascontiguousarray(Wp.T).astype(bf16)
    bpc = np.ascontiguousarray(bp.reshape(C, 1))
    in_maps = []
    for c in range(8):
        b, qh = c // 2, c % 2
        in_maps.append({
            "xqT": np.ascontiguousarray(xq[b, qh * NQ:(qh + 1) * NQ, :].T).astype(bf16),
            "xkT": np.ascontiguousarray(xk[b].T).astype(bf16),
            "wqT": wqT, "wkvT": wkvT, "wpT": wpT, "bp": bpc,
        })
    return in_maps


def gather(results):
    out_full = np.empty((N, B, C), np.float32)
    attn_full = np.empty((B, N, N), np.float32)
    for c in range(8):
        b, qh = c // 2, c % 2
        out_full[qh * NQ:(qh + 1) * NQ, b, :] = results[c]["outT"].T
        attn_full[b, qh * NQ:(qh + 1) * NQ, :] = results[c]["attnT"].T
    return out_full, attn_full


def kernel(xq, xk, xv, Wq, Wkv, Wp, bp):
    from concourse.bass_utils import run_bass_kernel_spmd

    if "nc" not in _CACHE:
        _CACHE["nc"] = build_nc()
    nc = _CACHE["nc"]
    in_maps = make_in_maps(
        np.asarray(xq, np.float32), np.asarray(xk, np.float32),
        np.asarray(Wq, np.float32), np.asarray(Wkv, np.float32),
        np.asarray(Wp, np.float32), np.asarray(bp, np.float32),
    )
    res = run_bass_kernel_spmd(nc, in_maps, core_ids=list(range(8)))
    return gather(res.results)


# revision 20
# speedup vs baseline: 1.1816x; 1.1816x over previous
"""Trainium2 Bass kernel for HPA-style attention (nn_Attention_33423435497672).

Reference computation (B=4, N=1024, C=1024, H=16, D=64):
    q  = xq @ Wq.T                      -> [B,N,C] -> heads [B,H,N,64]
    kv = xk @ Wkv.T ; k,v = split(kv)   -> [B,H,N,64] each  (xv unused)
    attn = (q @ k^T) * D**-0.5          -> [B,H,N,N]   (pre-softmax, saved)
    p = softmax(attn); x = p @ v        -> [B,N,C]
    out = x @ Wp.T + bp
    returns (out.transpose(1,0,2) [N,B,C], attn.sum(heads)/H [B,N,N])

Sharding: 8 cores = 4 batches x 2 query-halves.  Core c handles batch c//2,
query rows (c%2)*512..+512.  Each core computes the full K/V projection for
its batch (duplicated across the pair), so no collectives are needed; both
outputs partition cleanly by (batch, query-row) and the host reassembles.

On-device everything is kept transposed (contraction dim on partitions);
the host pre-transposes inputs and un-transposes outputs.  Matmuls run as
float32r (the full-rate fp32 PE path).  Softmax skips max-subtraction
(logits are ~N(0, 0.17), exp cannot overflow).

The per-head softmax denominator comes free from the AV matmul: v is stored
with 65 columns per head, the 65th column being all-ones, so row 64 of the
AV psum is sum_nk(exp).  The row is reciprocal'd and broadcast back across
partitions with a K=1 matmul against a ones row.
"""

import sys

sys.path.insert(0, "/opt/trn_rl_repo")

import numpy as np

import concourse.bass as bass
import concourse.mybir as mybir
from concourse import tile

B, N, C, H = 4, 1024, 1024, 16
D = C // H          # 64
NQ = N // 2         # 512 query rows per core
P = 128
F32 = mybir.dt.float32
BD = mybir.dt.bfloat16
SCALE = float(D) ** -0.5          # 0.125
ATTN_SCALE = SCALE / H            # 1/128

CT = C // P         # 8 c-tiles
NKT = N // P        # 8 nk-tiles
VW = 2 * D          # 128: 64 v columns + 64 ones columns per head
SCB = 2             # nk-chunks batched per scores psum tile

_CACHE = {}




def build_nc():
    nc = bass.Bass(target_bir_lowering=False)
    Exp = mybir.ActivationFunctionType.Exp

    xqT = nc.declare_dram_parameter("xqT", [C, NQ], BD, isOutput=False)
    xkT = nc.declare_dram_parameter("xkT", [C, N], BD, isOutput=False)
    wqT = nc.declare_dram_parameter("wqT", [C, C], BD, isOutput=False)
    wkvT = nc.declare_dram_parameter("wkvT", [C, 2 * C], BD, isOutput=False)
    wpT = nc.declare_dram_parameter("wpT", [C, C], BD, isOutput=False)
    bp = nc.declare_dram_parameter("bp", [C, 1], F32, isOutput=False)
    outT = nc.declare_dram_parameter("outT", [C, NQ], F32, isOutput=True)
    attnT = nc.declare_dram_parameter("attnT", [N, NQ], F32, isOutput=True)

    with nc.allow_low_precision(reason="bf16 compute path"), \
         tile.TileContext(nc) as tc:
        with (
            tc.tile_pool(name="consts", bufs=1) as consts,
            tc.tile_pool(name="acts", bufs=1) as acts,
            tc.tile_pool(name="wpool", bufs=1) as wpool,
            tc.tile_pool(name="stage", bufs=3) as stage,
            tc.tile_pool(name="scp", bufs=2, space="PSUM") as scp,
            tc.tile_pool(name="ppp", bufs=2, space="PSUM") as ppp,
            tc.tile_pool(name="avp", bufs=2, space="PSUM") as avp,
        ):
            bias_sb = consts.tile([P, CT], F32, name="bias", tag="bias")
            nc.sync.dma_start(bias_sb[:], bp.rearrange("(t p) o -> p (t o)", p=P))
            bias_warm = consts.tile([P, CT], F32, name="bias_warm", tag="bias_warm")
            nc.vector.tensor_copy(bias_warm[:], bias_sb[:])

            # ---- input activations (pre-transposed on host) ----
            xq_sb = [acts.tile([P, NQ], BD, name=f"xq{i}", tag=f"xq{i}") for i in range(CT)]
            xk_sb = [acts.tile([P, N], BD, name=f"xk{i}", tag=f"xk{i}") for i in range(CT)]
            for i in range(CT):
                nc.sync.dma_start(xq_sb[i][:], xqT[i * P:(i + 1) * P, :])

            qT_sb = [acts.tile([P, NQ], BD, name=f"qT{i}", tag=f"qT{i}") for i in range(CT)]
            kT_sb = [acts.tile([P, N], BD, name=f"kT{i}", tag=f"kT{i}") for i in range(CT)]
            v_sb = [acts.tile([P, H, VW], BD, name=f"v{i}", tag=f"v{i}") for i in range(NKT)]
            xT_sb = [acts.tile([P, NQ], BD, name=f"xT{i}", tag=f"xT{i}") for i in range(CT)]

            # ---- PE warmup: dummy matmuls cover the initial DMA stall and
            # get the HAM activity window hot before real work arrives ----
            dum = consts.tile([1, NQ], BD, name="dum", tag="dum")
            nc.vector.memset(dum[:], 0.0)
            for _ in range(3):
                wps = avp.tile([VW, NQ], F32, name="av", tag="av")
                for r in range(8):
                    nc.tensor.matmul(wps[0:1, :], dum[:, 0:1], dum[:],
                                     start=(r == 0), stop=(r == 7),
                                     skip_group_check=True)

            # ---- fused emission: projection / attention / attn-sum groups
            # interleaved so the PE never idles long enough to re-throttle ----
            wq_t = [wpool.tile([P, C], BD, name=f"wq_t{i}", tag=f"wq{i}") for i in range(CT)]
            for ct in range(CT):
                nc.sync.dma_start(wq_t[ct][:], wqT[ct * P:(ct + 1) * P, :])
            wk_t = [wpool.tile([P, C], BD, name=f"wk_t{i}", tag=f"wk{i}") for i in range(CT)]
            for ct in range(CT):
                nc.sync.dma_start(wk_t[ct][:], wkvT[ct * P:(ct + 1) * P, 0:C])
            for i in range(CT):
                nc.sync.dma_start(xk_sb[i][:], xkT[i * P:(i + 1) * P, :])
            wv_t = [wpool.tile([P, C], BD, name=f"wv_t{i}", tag=f"wv{i}") for i in range(CT)]
            for ct in range(CT):
                nc.sync.dma_start(wv_t[ct][:], wkvT[ct * P:(ct + 1) * P, C:2 * C])

            def emit_A(ot):
                ps = ppp.tile([P, NQ], F32, name="pp", tag="pp")
                for ct in range(CT):
                    nc.tensor.matmul(
                        ps[:], wq_t[ct][:, ot * P:(ot + 1) * P], xq_sb[ct][:],
                        start=(ct == 0), stop=(ct == CT - 1))
                nc.vector.tensor_copy(qT_sb[ot][:], ps[:])

            def emit_B1(ot, hf):
                ps = ppp.tile([P, NQ], F32, name="pp", tag="pp")
                for ct in range(CT):
                    nc.tensor.matmul(
                        ps[:], wk_t[ct][:, ot * P:(ot + 1) * P],
                        xk_sb[ct][:, hf * NQ:(hf + 1) * NQ],
                        start=(ct == 0), stop=(ct == CT - 1))
                nc.vector.tensor_copy(kT_sb[ot][:, hf * NQ:(hf + 1) * NQ], ps[:])

            def emit_B2(nt, hf):
                ps = ppp.tile([P, NQ], F32, name="pp", tag="pp")
                for ct in range(CT):
                    nc.tensor.matmul(
                        ps[:], xk_sb[ct][:, nt * P:(nt + 1) * P],
                        wv_t[ct][:, hf * NQ:(hf + 1) * NQ],
                        start=(ct == 0), stop=(ct == CT - 1))
                nc.vector.tensor_copy(
                    v_sb[nt][:, hf * (H // 2):(hf + 1) * (H // 2), 0:D],
                    ps.rearrange("p (h d) -> p h d", d=D))

            def emit_E(nt):
                ps = ppp.tile([P, NQ], F32, name="pp", tag="pp")
                for ct in range(CT):
                    nc.tensor.matmul(
                        ps[:], kT_sb[ct][:, nt * P:(nt + 1) * P], qT_sb[ct][:],
                        start=(ct == 0), stop=(ct == CT - 1))
                ast = stage.tile([P, NQ], F32, name=f"ast{nt}", tag=f"ast{nt}", bufs=1)
                nc.vector.tensor_scalar_mul(ast[:], ps[:], ATTN_SCALE)
                nc.sync.dma_start(attnT[nt * P:(nt + 1) * P, :], ast[:])

            def emit_head(h, inner=None):
                ht, hr = h // 2, (h % 2) * D
                av = avp.tile([VW, NQ], F32, name="av", tag="av")
                for j in range(NKT // SCB):
                    sc = scp.tile([P, SCB, NQ], F32, name="sc", tag="sc")
                    for tt in range(SCB):
                        t = j * SCB + tt
                        nc.tensor.matmul(
                            sc[:, tt, :],
                            kT_sb[ht][hr:hr + D, t * P:(t + 1) * P],
                            qT_sb[ht][hr:hr + D, :],
                            start=True, stop=True)
                    ex = acts.tile([P, SCB, NQ], BD, name=f"ex{j}", tag=f"ex{j}")
                    nc.scalar.activation(ex[:], sc[:], Exp, scale=SCALE)
                    for f in (inner or {}).get(j, []):
                        f()
                    for tt in range(SCB):
                        t = j * SCB + tt
                        nc.tensor.matmul(
                            av[:], v_sb[t][:, h, :], ex[:, tt, :],
                            start=(t == 0), stop=(t == NKT - 1),
                            skip_group_check=True)
                rec = stage.tile([D, NQ], F32, name="rec", tag="rec")
                nc.vector.reciprocal(rec[:, 0:NQ // 2], av[D:VW, 0:NQ // 2])
                nc.vector.reciprocal(rec[:, NQ // 2:], av[D:VW, NQ // 2:])
                nc.vector.tensor_mul(xT_sb[ht][hr:hr + D, :], av[0:D, :], rec[:])

            # prelude: just what head 0's QK needs; v ones columns; the
            # hf=0 V-projection groups run inside head 0 between exp and AV
            for nt in range(NKT):
                nc.vector.memset(v_sb[nt][:, :, D:VW], 1.0)
            emit_A(0)
            emit_B1(0, 0)
            emit_B1(0, 1)

            head0_inner = {
                0: [lambda: emit_B2(0, 0), lambda: emit_B2(1, 0)],
                1: [lambda: emit_B2(2, 0), lambda: emit_B2(3, 0)],
                2: [lambda: emit_B2(4, 0), lambda: emit_B2(5, 0)],
                3: [lambda: emit_B2(6, 0), lambda: emit_B2(7, 0)],
            }
            fillers = {
                0: [lambda: emit_A(1), lambda: emit_B1(1, 0), lambda: emit_B1(1, 1)],
                1: [lambda: emit_B2(0, 1), lambda: emit_B2(1, 1), lambda: emit_A(2)],
                2: [lambda: emit_B1(2, 0), lambda: emit_B1(2, 1), lambda: emit_B2(2, 1)],
                3: [lambda: emit_B2(3, 1), lambda: emit_B2(4, 1), lambda: emit_A(3)],
                4: [lambda: emit_B1(3, 0), lambda: emit_B1(3, 1), lambda: emit_B2(5, 1)],
                5: [lambda: emit_B2(6, 1), lambda: emit_B2(7, 1), lambda: emit_A(4)],
                6: [lambda: emit_B1(4, 0), lambda: emit_B1(4, 1), lambda: emit_A(5)],
                7: [lambda: emit_B1(5, 0), lambda: emit_B1(5, 1)],
                8: [lambda: emit_A(6), lambda: emit_B1(6, 0)],
                9: [lambda: emit_B1(6, 1), lambda: emit_A(7)],
                10: [lambda: emit_B1(7, 0), lambda: emit_B1(7, 1)],
                12: [lambda: emit_E(0), lambda: emit_E(1)],
                13: [lambda: emit_E(2), lambda: emit_E(3)],
                14: [lambda: emit_E(4), lambda: emit_E(5)],
                15: [lambda: emit_E(6), lambda: emit_E(7)],
            }
            for h in range(H):
                emit_head(h, inner=head0_inner if h == 0 else None)
                for f in fillers.get(h, []):
                    f()

            # ---- stage D: outT[co, nq] = sum_ci wpT[ci, co] xT[ci, nq] + bp ----
            wp_t = [wpool.tile([P, C], BD, name=f"wp_t{i}", tag=f"wq{i}") for i in range(CT)]
            for ct in range(CT):
                nc.sync.dma_start(wp_t[ct][:], wpT[ct * P:(ct + 1) * P, :])
            for ot in range(CT):
                ps = ppp.tile([P, NQ], F32, name="pp", tag="pp")
                for ct in range(CT):
                    nc.tensor.matmul(
                        ps[:], wp_t[ct][:, ot * P:(ot + 1) * P], xT_sb[ct][:],
                        start=(ct == 0), stop=(ct == CT - 1))
                ost = stage.tile([P, NQ], F32, name=f"ost{ot}", tag=f"ost{ot}", bufs=1)
                nc.vector.tensor_scalar_add(ost[:], ps[:], bias_sb[:, ot:ot + 1])
                nc.sync.dma_start(outT[ot * P:(ot + 1) * P, :], ost[:])

    _fix_wait_overflow(nc)
    nc.finalize()
    return nc


def _fix_wait_overflow(nc):
    """Walrus's per-instruction ISA structs carry a single sync-wait slot,
    but Tile sometimes attaches two or three.  Three sound repairs:
    - DMA instructions: drop waits on a queue semaphore the instruction
      itself increments (per-engine descriptor FIFO makes them implicit);
    - compute engines: drop own-semaphore waits whose producing instruction
      retired >=3 instructions earlier on the same in-order engine;
    - matmuls: move leftover extra waits onto the immediately-preceding
      Ldweights (no waits, no updates, so no cycle risk)."""
    skip = ("InstDrain", "InstEventSemaphore")
    # Split over-subscribed tail drains: one wait per InstDrain.
    for block in nc.m.functions[0].blocks:
        edits = []
        for idx, inst in enumerate(block.instructions):
            si = getattr(inst, "sync_info", None)
            if (inst.__class__.__name__ == "InstDrain" and si is not None
                    and len(si.on_wait) > 1):
                extra = []
                while len(si.on_wait) > 1:
                    extra.append(si.on_wait.pop())
                pres = []
                for w in extra:
                    d = mybir.InstDrain(
                        name=nc.get_next_instruction_name(),
                        ins=[], outs=[], bass_is_fusable=False)
                    d.engine = inst.engine
                    d.sync_info = mybir.SyncInfo(on_wait=[w], on_update=[])
                    pres.append(d)
                edits.append((idx, pres))
        for idx, pres in reversed(edits):
            for d in reversed(pres):
                block.instructions.insert(idx, d)
    for block in nc.m.functions[0].blocks:
        pos_by_eng = {}
        prev_by_eng = {}
        inc_hist = {}      # (eng, sem) -> [(stream_pos, cum_after)]
        for inst in block.instructions:
            eng = str(getattr(inst, "engine", None))
            pos = pos_by_eng.get(eng, 0)
            si = getattr(inst, "sync_info", None)
            cls = inst.__class__.__name__
            if si is not None and len(si.on_wait) > 1 and cls not in skip:
                ups = {u.ant_name for u in si.on_update}
                keep = []
                for w in si.on_wait:
                    nm = getattr(w, "ant_name", "") or ""
                    if nm in ups and w.wait_value is not None:
                        if cls == "InstDMACopy":
                            continue                      # FIFO-implied
                        hist = inc_hist.get((eng, nm), [])
                        idx = next((p for p, cum in hist
                                    if cum >= w.wait_value), None)
                        if idx is not None and pos - idx - 1 >= 3:
                            continue                      # long retired
                    keep.append(w)
                while len(si.on_wait) > 0:
                    si.on_wait.pop()
                for w in keep:
                    si.on_wait.append(w)
                if len(si.on_wait) > 1:
                    prev = prev_by_eng.get(eng)
                    psi = prev.sync_info if prev is not None else None
                    if psi is None and prev is not None:
                        psi = mybir.SyncInfo(on_wait=[], on_update=[])
                        prev.sync_info = psi
                    if (psi is not None and len(psi.on_wait) == 0
                            and len(psi.on_update) == 0):
                        while len(si.on_wait) > 1:
                            psi.on_wait.append(si.on_wait.pop())
                assert len(si.on_wait) <= 1, (
                    f"{inst.name} ({cls}): still "
                    f"{[(w.ant_name, w.wait_value) for w in si.on_wait]}")
            if si is not None:
                for u in si.on_update:
                    key = (eng, u.ant_name)
                    hist = inc_hist.setdefault(key, [])
                    cum = hist[-1][1] if hist else 0
                    hist.append((pos, cum + (u.update_value or 0)))
            prev_by_eng[eng] = inst
            pos_by_eng[eng] = pos + 1


def make_in_maps(xq, xk, Wq, Wkv, Wp, bp):
    bf16 = mybir.dt.np(BD)
    wqT = np.ascontiguousarray(Wq.T).astype(bf16)
    wkvT = np.ascontiguousarray(Wkv.T).astype(bf16)
    wpT = np.ascontiguousarray(Wp.T).astype(bf16)
    bpc = np.ascontiguousarray(bp.reshape(C, 1))
    in_maps = []
    for c in range(8):
        b, qh = c // 2, c % 2
        in_maps.append({
            "xqT": np.ascontiguousarray(xq[b, qh * NQ:(qh + 1) * NQ, :].T).astype(bf16),
            "xkT": np.ascontiguousarray(xk[b].T).astype(bf16),
            "wqT": wqT, "wkvT": wkvT, "wpT": wpT, "bp": bpc,
        })
    return in_maps


def gather(results):
    out_full = np.empty((N, B, C), np.float32)
    attn_full = np.empty((B, N, N), np.float32)
    for c in range(8):
        b, qh = c // 2, c % 2
        out_full[qh * NQ:(qh + 1) * NQ, b, :] = results[c]["outT"].T
        attn_full[b, qh * NQ:(qh + 1) * NQ, :] = results[c]["attnT"].T
    return out_full, attn_full


def kernel(xq, xk, xv, Wq, Wkv, Wp, bp):
    from concourse.bass_utils import run_bass_kernel_spmd

    if "nc" not in _CACHE:
        _CACHE["nc"] = build_nc()
    nc = _CACHE["nc"]
    in_maps = make_in_maps(
        np.asarray(xq, np.float32), np.asarray(xk, np.float32),
        np.asarray(Wq, np.float32), np.asarray(Wkv, np.float32),
        np.asarray(Wp, np.float32), np.asarray(bp, np.float32),
    )
    res = run_bass_kernel_spmd(nc, in_maps, core_ids=list(range(8)))
    return gather(res.results)


# revision 21
# speedup vs baseline: 1.1954x; 1.0116x over previous
"""Trainium2 Bass kernel for HPA-style attention (nn_Attention_33423435497672).

Reference computation (B=4, N=1024, C=1024, H=16, D=64):
    q  = xq @ Wq.T                      -> [B,N,C] -> heads [B,H,N,64]
    kv = xk @ Wkv.T ; k,v = split(kv)   -> [B,H,N,64] each  (xv unused)
    attn = (q @ k^T) * D**-0.5          -> [B,H,N,N]   (pre-softmax, saved)
    p = softmax(attn); x = p @ v        -> [B,N,C]
    out = x @ Wp.T + bp
    returns (out.transpose(1,0,2) [N,B,C], attn.sum(heads)/H [B,N,N])

Sharding: 8 cores = 4 batches x 2 query-halves.  Core c handles batch c//2,
query rows (c%2)*512..+512.  Each core computes the full K/V projection for
its batch (duplicated across the pair), so no collectives are needed; both
outputs partition cleanly by (batch, query-row) and the host reassembles.

On-device everything is kept transposed (contraction dim on partitions);
the host pre-transposes inputs and un-transposes outputs.  Matmuls run as
float32r (the full-rate fp32 PE path).  Softmax skips max-subtraction
(logits are ~N(0, 0.17), exp cannot overflow).

The per-head softmax denominator comes free from the AV matmul: v is stored
with 65 columns per head, the 65th column being all-ones, so row 64 of the
AV psum is sum_nk(exp).  The row is reciprocal'd and broadcast back across
partitions with a K=1 matmul against a ones row.
"""

import sys

sys.path.insert(0, "/opt/trn_rl_repo")

import numpy as np

import concourse.bass as bass
import concourse.mybir as mybir
from concourse import tile

B, N, C, H = 4, 1024, 1024, 16
D = C // H          # 64
NQ = N // 2         # 512 query rows per core
P = 128
F32 = mybir.dt.float32
BD = mybir.dt.bfloat16
SCALE = float(D) ** -0.5          # 0.125
ATTN_SCALE = SCALE / H            # 1/128

CT = C // P         # 8 c-tiles
NKT = N // P        # 8 nk-tiles
VW = 2 * D          # 128: 64 v columns + 64 ones columns per head
SCB = 2             # nk-chunks batched per scores psum tile

_CACHE = {}




def build_nc():
    nc = bass.Bass(target_bir_lowering=False)
    Exp = mybir.ActivationFunctionType.Exp

    xqT = nc.declare_dram_parameter("xqT", [C, NQ], BD, isOutput=False)
    xkT = nc.declare_dram_parameter("xkT", [C, N], BD, isOutput=False)
    wqT = nc.declare_dram_parameter("wqT", [C, C], BD, isOutput=False)
    wkvT = nc.declare_dram_parameter("wkvT", [C, 2 * C], BD, isOutput=False)
    wpT = nc.declare_dram_parameter("wpT", [C, C], BD, isOutput=False)
    bp = nc.declare_dram_parameter("bp", [C, 1], F32, isOutput=False)
    outT = nc.declare_dram_parameter("outT", [C, NQ], F32, isOutput=True)
    attnT = nc.declare_dram_parameter("attnT", [N, NQ], F32, isOutput=True)

    with nc.allow_low_precision(reason="bf16 compute path"), \
         tile.TileContext(nc) as tc:
        with (
            tc.tile_pool(name="consts", bufs=1) as consts,
            tc.tile_pool(name="acts", bufs=1) as acts,
            tc.tile_pool(name="wpool", bufs=1) as wpool,
            tc.tile_pool(name="stage", bufs=3) as stage,
            tc.tile_pool(name="scp", bufs=2, space="PSUM") as scp,
            tc.tile_pool(name="ppp", bufs=2, space="PSUM") as ppp,
            tc.tile_pool(name="avp", bufs=2, space="PSUM") as avp,
        ):
            bias_sb = consts.tile([P, CT], F32, name="bias", tag="bias")
            nc.sync.dma_start(bias_sb[:], bp.rearrange("(t p) o -> p (t o)", p=P))
            bias_warm = consts.tile([P, CT], F32, name="bias_warm", tag="bias_warm")
            nc.vector.tensor_copy(bias_warm[:], bias_sb[:])

            # ---- input activations (pre-transposed on host) ----
            xq_sb = [acts.tile([P, NQ], BD, name=f"xq{i}", tag=f"xq{i}") for i in range(CT)]
            xk_sb = [acts.tile([P, N], BD, name=f"xk{i}", tag=f"xk{i}") for i in range(CT)]
            for i in range(CT):
                nc.sync.dma_start(xq_sb[i][:], xqT[i * P:(i + 1) * P, :])

            qT_sb = [acts.tile([P, NQ], BD, name=f"qT{i}", tag=f"qT{i}") for i in range(CT)]
            kT_sb = [acts.tile([P, N], BD, name=f"kT{i}", tag=f"kT{i}") for i in range(CT)]
            v_sb = [acts.tile([P, H, VW], BD, name=f"v{i}", tag=f"v{i}") for i in range(NKT)]
            xT_sb = [acts.tile([P, NQ], BD, name=f"xT{i}", tag=f"xT{i}") for i in range(CT)]

            # ---- PE warmup: dummy matmuls cover the initial DMA stall and
            # get the HAM activity window hot before real work arrives ----
            dum = consts.tile([1, NQ], BD, name="dum", tag="dum")
            nc.vector.memset(dum[:], 0.0)
            for _ in range(3):
                wps = avp.tile([VW, NQ], F32, name="av", tag="av")
                for r in range(8):
                    nc.tensor.matmul(wps[0:1, :], dum[:, 0:1], dum[:],
                                     start=(r == 0), stop=(r == 7),
                                     skip_group_check=True)

            # ---- fused emission: projection / attention / attn-sum groups
            # interleaved so the PE never idles long enough to re-throttle ----
            wq_t = [wpool.tile([P, C], BD, name=f"wq_t{i}", tag=f"wq{i}") for i in range(CT)]
            for ct in range(CT):
                nc.sync.dma_start(wq_t[ct][:], wqT[ct * P:(ct + 1) * P, :])
            wk_t = [wpool.tile([P, C], BD, name=f"wk_t{i}", tag=f"wk{i}") for i in range(CT)]
            for ct in range(CT):
                nc.sync.dma_start(wk_t[ct][:], wkvT[ct * P:(ct + 1) * P, 0:C])
            for i in range(CT):
                nc.sync.dma_start(xk_sb[i][:], xkT[i * P:(i + 1) * P, :])
            wv_t = [wpool.tile([P, C], BD, name=f"wv_t{i}", tag=f"wv{i}") for i in range(CT)]
            for ct in range(CT):
                nc.sync.dma_start(wv_t[ct][:], wkvT[ct * P:(ct + 1) * P, C:2 * C])

            def emit_A(ot):
                ps = ppp.tile([P, NQ], F32, name="pp", tag="pp")
                for ct in range(CT):
                    nc.tensor.matmul(
                        ps[:], wq_t[ct][:, ot * P:(ot + 1) * P], xq_sb[ct][:],
                        start=(ct == 0), stop=(ct == CT - 1))
                nc.vector.tensor_copy(qT_sb[ot][:], ps[:])

            def emit_B1(ot, hf):
                ps = ppp.tile([P, NQ], F32, name="pp", tag="pp")
                for ct in range(CT):
                    nc.tensor.matmul(
                        ps[:], wk_t[ct][:, ot * P:(ot + 1) * P],
                        xk_sb[ct][:, hf * NQ:(hf + 1) * NQ],
                        start=(ct == 0), stop=(ct == CT - 1))
                nc.vector.tensor_copy(kT_sb[ot][:, hf * NQ:(hf + 1) * NQ], ps[:])

            def emit_B2(nt, hf):
                ps = ppp.tile([P, NQ], F32, name="pp", tag="pp")
                for ct in range(CT):
                    nc.tensor.matmul(
                        ps[:], xk_sb[ct][:, nt * P:(nt + 1) * P],
                        wv_t[ct][:, hf * NQ:(hf + 1) * NQ],
                        start=(ct == 0), stop=(ct == CT - 1))
                nc.vector.tensor_copy(
                    v_sb[nt][:, hf * (H // 2):(hf + 1) * (H // 2), 0:D],
                    ps.rearrange("p (h d) -> p h d", d=D))

            def emit_E(nt):
                ps = ppp.tile([P, NQ], F32, name="pp", tag="pp")
                for ct in range(CT):
                    nc.tensor.matmul(
                        ps[:], kT_sb[ct][:, nt * P:(nt + 1) * P], qT_sb[ct][:],
                        start=(ct == 0), stop=(ct == CT - 1))
                ast = stage.tile([P, NQ], F32, name=f"ast{nt}", tag=f"ast{nt}", bufs=1)
                nc.vector.tensor_scalar_mul(ast[:], ps[:], ATTN_SCALE)
                nc.sync.dma_start(attnT[nt * P:(nt + 1) * P, :], ast[:])

            def emit_head(h, inner=None):
                ht, hr = h // 2, (h % 2) * D
                av = avp.tile([VW, NQ], F32, name="av", tag="av")
                for j in range(NKT // SCB):
                    sc = scp.tile([P, SCB, NQ], F32, name="sc", tag="sc")
                    for tt in range(SCB):
                        t = j * SCB + tt
                        nc.tensor.matmul(
                            sc[:, tt, :],
                            kT_sb[ht][hr:hr + D, t * P:(t + 1) * P],
                            qT_sb[ht][hr:hr + D, :],
                            start=True, stop=True)
                    ex = acts.tile([P, SCB, NQ], BD, name=f"ex{j}", tag=f"ex{j}")
                    nc.scalar.activation(ex[:], sc[:], Exp, scale=SCALE)
                    for f in (inner or {}).get(j, []):
                        f()
                    for tt in range(SCB):
                        t = j * SCB + tt
                        nc.tensor.matmul(
                            av[:], v_sb[t][:, h, :], ex[:, tt, :],
                            start=(t == 0), stop=(t == NKT - 1),
                            skip_group_check=True)
                rec = stage.tile([D, NQ], F32, name="rec", tag="rec")
                nc.vector.reciprocal(rec[:, 0:NQ // 2], av[D:VW, 0:NQ // 2])
                nc.vector.reciprocal(rec[:, NQ // 2:], av[D:VW, NQ // 2:])
                nc.vector.tensor_mul(xT_sb[ht][hr:hr + D, :], av[0:D, :], rec[:])

            # prelude: just what head 0's QK needs; v ones columns; the
            # hf=0 V-projection groups run inside head 0 between exp and AV
            for nt in range(NKT):
                nc.vector.memset(v_sb[nt][:, :, D:VW], 1.0)
            emit_A(0)
            emit_B1(0, 0)
            emit_B1(0, 1)

            def emit_head0():
                ht, hr = 0, 0
                av = avp.tile([VW, NQ], F32, name="av", tag="av")
                exs = []
                for j in range(NKT // SCB):
                    sc = scp.tile([P, SCB, NQ], F32, name="sc", tag="sc")
                    for tt in range(SCB):
                        t = j * SCB + tt
                        nc.tensor.matmul(
                            sc[:, tt, :],
                            kT_sb[ht][hr:hr + D, t * P:(t + 1) * P],
                            qT_sb[ht][hr:hr + D, :],
                            start=True, stop=True)
                    ex = acts.tile([P, SCB, NQ], BD, name=f"ex{j}", tag=f"ex{j}")
                    nc.scalar.activation(ex[:], sc[:], Exp, scale=SCALE)
                    exs.append(ex)
                emit_A(1)
                emit_B1(1, 0)
                emit_B1(1, 1)
                for nt in range(NKT):
                    emit_B2(nt, 0)
                for j in range(NKT // SCB):
                    for tt in range(SCB):
                        t = j * SCB + tt
                        nc.tensor.matmul(
                            av[:], v_sb[t][:, 0, :], exs[j][:, tt, :],
                            start=(t == 0), stop=(t == NKT - 1),
                            skip_group_check=True)
                rec = stage.tile([D, NQ], F32, name="rec", tag="rec")
                nc.vector.reciprocal(rec[:, 0:NQ // 2], av[D:VW, 0:NQ // 2])
                nc.vector.reciprocal(rec[:, NQ // 2:], av[D:VW, NQ // 2:])
                nc.vector.tensor_mul(xT_sb[0][0:D, :], av[0:D, :], rec[:])
            fillers = {
                0: [lambda: emit_B2(0, 1), lambda: emit_B2(1, 1)],
                1: [lambda: emit_A(2), lambda: emit_B2(2, 1), lambda: emit_B2(3, 1)],
                2: [lambda: emit_B1(2, 0), lambda: emit_B1(2, 1), lambda: emit_B2(4, 1)],
                3: [lambda: emit_A(3), lambda: emit_B2(5, 1), lambda: emit_B2(6, 1)],
                4: [lambda: emit_B1(3, 0), lambda: emit_B1(3, 1), lambda: emit_B2(7, 1)],
                5: [lambda: emit_A(4), lambda: emit_B1(4, 0)],
                6: [lambda: emit_B1(4, 1), lambda: emit_A(5)],
                7: [lambda: emit_B1(5, 0), lambda: emit_B1(5, 1)],
                8: [lambda: emit_A(6), lambda: emit_B1(6, 0)],
                9: [lambda: emit_B1(6, 1), lambda: emit_A(7)],
                10: [lambda: emit_B1(7, 0), lambda: emit_B1(7, 1)],
                12: [lambda: emit_E(0), lambda: emit_E(1)],
                13: [lambda: emit_E(2), lambda: emit_E(3)],
                14: [lambda: emit_E(4), lambda: emit_E(5)],
                15: [lambda: emit_E(6)],
            }
            for h in range(H):
                if h == 0:
                    emit_head0()
                else:
                    emit_head(h)
                for f in fillers.get(h, []):
                    f()

            # ---- stage D: outT[co, nq] = sum_ci wpT[ci, co] xT[ci, nq] + bp ----
            wp_t = [wpool.tile([P, C], BD, name=f"wp_t{i}", tag=f"wq{i}") for i in range(CT)]
            for ct in range(CT):
                nc.sync.dma_start(wp_t[ct][:], wpT[ct * P:(ct + 1) * P, :])
            for ot in range(CT):
                ps = ppp.tile([P, NQ], F32, name="pp", tag="pp")
                for ct in range(CT):
                    nc.tensor.matmul(
                        ps[:], wp_t[ct][:, ot * P:(ot + 1) * P], xT_sb[ct][:],
                        start=(ct == 0), stop=(ct == CT - 1))
                ost = stage.tile([P, NQ], F32, name=f"ost{ot}", tag=f"ost{ot}", bufs=1)
                nc.vector.tensor_scalar_add(ost[:], ps[:], bias_sb[:, ot:ot + 1])
                nc.sync.dma_start(outT[ot * P:(ot + 1) * P, :], ost[:])
                if ot == 1:
                    emit_E(7)

    _fix_wait_overflow(nc)
    nc.finalize()
    return nc


def _fix_wait_overflow(nc):
    """Walrus's per-instruction ISA structs carry a single sync-wait slot,
    but Tile sometimes attaches two or three.  Three sound repairs:
    - DMA instructions: drop waits on a queue semaphore the instruction
      itself increments (per-engine descriptor FIFO makes them implicit);
    - compute engines: drop own-semaphore waits whose producing instruction
      retired >=3 instructions earlier on the same in-order engine;
    - matmuls: move leftover extra waits onto the immediately-preceding
      Ldweights (no waits, no updates, so no cycle risk)."""
    skip = ("InstDrain", "InstEventSemaphore")
    # Split over-subscribed tail drains: one wait per InstDrain.
    for block in nc.m.functions[0].blocks:
        edits = []
        for idx, inst in enumerate(block.instructions):
            si = getattr(inst, "sync_info", None)
            if (inst.__class__.__name__ == "InstDrain" and si is not None
                    and len(si.on_wait) > 1):
                extra = []
                while len(si.on_wait) > 1:
                    extra.append(si.on_wait.pop())
                pres = []
                for w in extra:
                    d = mybir.InstDrain(
                        name=nc.get_next_instruction_name(),
                        ins=[], outs=[], bass_is_fusable=False)
                    d.engine = inst.engine
                    d.sync_info = mybir.SyncInfo(on_wait=[w], on_update=[])
                    pres.append(d)
                edits.append((idx, pres))
        for idx, pres in reversed(edits):
            for d in reversed(pres):
                block.instructions.insert(idx, d)
    for block in nc.m.functions[0].blocks:
        pos_by_eng = {}
        prev_by_eng = {}
        inc_hist = {}      # (eng, sem) -> [(stream_pos, cum_after)]
        for inst in block.instructions:
            eng = str(getattr(inst, "engine", None))
            pos = pos_by_eng.get(eng, 0)
            si = getattr(inst, "sync_info", None)
            cls = inst.__class__.__name__
            if si is not None and len(si.on_wait) > 1 and cls not in skip:
                ups = {u.ant_name for u in si.on_update}
                keep = []
                for w in si.on_wait:
                    nm = getattr(w, "ant_name", "") or ""
                    if nm in ups and w.wait_value is not None:
                        if cls == "InstDMACopy":
                            continue                      # FIFO-implied
                        hist = inc_hist.get((eng, nm), [])
                        idx = next((p for p, cum in hist
                                    if cum >= w.wait_value), None)
                        if idx is not None and pos - idx - 1 >= 3:
                            continue                      # long retired
                    keep.append(w)
                while len(si.on_wait) > 0:
                    si.on_wait.pop()
                for w in keep:
                    si.on_wait.append(w)
                if len(si.on_wait) > 1:
                    prev = prev_by_eng.get(eng)
                    psi = prev.sync_info if prev is not None else None
                    if psi is None and prev is not None:
                        psi = mybir.SyncInfo(on_wait=[], on_update=[])
                        prev.sync_info = psi
                    if (psi is not None and len(psi.on_wait) == 0
                            and len(psi.on_update) == 0):
                        while len(si.on_wait) > 1:
                            psi.on_wait.append(si.on_wait.pop())
                assert len(si.on_wait) <= 1, (
                    f"{inst.name} ({cls}): still "
                    f"{[(w.ant_name, w.wait_value) for w in si.on_wait]}")
            if si is not None:
                for u in si.on_update:
                    key = (eng, u.ant_name)
                    hist = inc_hist.setdefault(key, [])
                    cum = hist[-1][1] if hist else 0
                    hist.append((pos, cum + (u.update_value or 0)))
            prev_by_eng[eng] = inst
            pos_by_eng[eng] = pos + 1


def make_in_maps(xq, xk, Wq, Wkv, Wp, bp):
    bf16 = mybir.dt.np(BD)
    wqT = np.ascontiguousarray(Wq.T).astype(bf16)
    wkvT = np.ascontiguousarray(Wkv.T).astype(bf16)
    wpT = np.ascontiguousarray(Wp.T).astype(bf16)
    bpc = np.ascontiguousarray(bp.reshape(C, 1))
    in_maps = []
    for c in range(8):
        b, qh = c // 2, c % 2
        in_maps.append({
            "xqT": np.ascontiguousarray(xq[b, qh * NQ:(qh + 1) * NQ, :].T).astype(bf16),
            "xkT": np.ascontiguousarray(xk[b].T).astype(bf16),
            "wqT": wqT, "wkvT": wkvT, "wpT": wpT, "bp": bpc,
        })
    return in_maps


def gather(results):
    out_full = np.empty((N, B, C), np.float32)
    attn_full = np.empty((B, N, N), np.float32)
    for c in range(8):
        b, qh = c // 2, c % 2
        out_full[qh * NQ:(qh + 1) * NQ, b, :] = results[c]["outT"].T
        attn_full[b, qh * NQ:(qh + 1) * NQ, :] = results[c]["attnT"].T
    return out_full, attn_full


def kernel(xq, xk, xv, Wq, Wkv, Wp, bp):
    from concourse.bass_utils import run_bass_kernel_spmd

    if "nc" not in _CACHE:
        _CACHE["nc"] = build_nc()
    nc = _CACHE["nc"]
    in_maps = make_in_maps(
        np.asarray(xq, np.float32), np.asarray(xk, np.float32),
        np.asarray(Wq, np.float32), np.asarray(Wkv, np.float32),
        np.asarray(Wp, np.float32), np.asarray(bp, np.float32),
    )
    res = run_bass_kernel_spmd(nc, in_maps, core_ids=list(range(8)))
    return gather(res.results)


# revision 24
# speedup vs baseline: 1.2046x; 1.0077x over previous
"""Trainium2 Bass kernel for HPA-style attention (nn_Attention_33423435497672).

Reference computation (B=4, N=1024, C=1024, H=16, D=64):
    q  = xq @ Wq.T                      -> [B,N,C] -> heads [B,H,N,64]
    kv = xk @ Wkv.T ; k,v = split(kv)   -> [B,H,N,64] each  (xv unused)
    attn = (q @ k^T) * D**-0.5          -> [B,H,N,N]   (pre-softmax, saved)
    p = softmax(attn); x = p @ v        -> [B,N,C]
    out = x @ Wp.T + bp
    returns (out.transpose(1,0,2) [N,B,C], attn.sum(heads)/H [B,N,N])

Sharding: 8 cores = 4 batches x 2 query-halves.  Core c handles batch c//2,
query rows (c%2)*512..+512.  Each core computes the full K/V projection for
its batch (duplicated across the pair), so no collectives are needed; both
outputs partition cleanly by (batch, query-row) and the host reassembles.

On-device everything is kept transposed (contraction dim on partitions);
the host pre-transposes inputs and un-transposes outputs.  Matmuls run as
float32r (the full-rate fp32 PE path).  Softmax skips max-subtraction
(logits are ~N(0, 0.17), exp cannot overflow).

The per-head softmax denominator comes free from the AV matmul: v is stored
with 65 columns per head, the 65th column being all-ones, so row 64 of the
AV psum is sum_nk(exp).  The row is reciprocal'd and broadcast back across
partitions with a K=1 matmul against a ones row.
"""

import sys

sys.path.insert(0, "/opt/trn_rl_repo")

import numpy as np

import concourse.bass as bass
import concourse.mybir as mybir
from concourse import tile

B, N, C, H = 4, 1024, 1024, 16
D = C // H          # 64
NQ = N // 2         # 512 query rows per core
P = 128
F32 = mybir.dt.float32
BD = mybir.dt.bfloat16
SCALE = float(D) ** -0.5          # 0.125
ATTN_SCALE = SCALE / H            # 1/128

CT = C // P         # 8 c-tiles
NKT = N // P        # 8 nk-tiles
VW = 2 * D          # 128: 64 v columns + 64 ones columns per head
SCB = 2             # nk-chunks batched per scores psum tile

_CACHE = {}




def build_nc():
    nc = bass.Bass(target_bir_lowering=False)
    Exp = mybir.ActivationFunctionType.Exp
    Cpy = mybir.ActivationFunctionType.Identity

    xqT = nc.declare_dram_parameter("xqT", [C, NQ], BD, isOutput=False)
    xkT = nc.declare_dram_parameter("xkT", [C, N], BD, isOutput=False)
    wqT = nc.declare_dram_parameter("wqT", [C, C], BD, isOutput=False)
    wkvT = nc.declare_dram_parameter("wkvT", [C, 2 * C], BD, isOutput=False)
    wpT = nc.declare_dram_parameter("wpT", [C, C], BD, isOutput=False)
    bp = nc.declare_dram_parameter("bp", [C, 1], F32, isOutput=False)
    outT = nc.declare_dram_parameter("outT", [C, NQ], F32, isOutput=True)
    attnT = nc.declare_dram_parameter("attnT", [N, NQ], F32, isOutput=True)

    with nc.allow_low_precision(reason="bf16 compute path"), \
         tile.TileContext(nc) as tc:
        with (
            tc.tile_pool(name="consts", bufs=1) as consts,
            tc.tile_pool(name="acts", bufs=1) as acts,
            tc.tile_pool(name="wpool", bufs=1) as wpool,
            tc.tile_pool(name="stage", bufs=3) as stage,
            tc.tile_pool(name="scp", bufs=2, space="PSUM") as scp,
            tc.tile_pool(name="ppp", bufs=2, space="PSUM") as ppp,
            tc.tile_pool(name="avp", bufs=2, space="PSUM") as avp,
        ):
            bias_sb = consts.tile([P, CT], F32, name="bias", tag="bias")
            nc.sync.dma_start(bias_sb[:], bp.rearrange("(t p) o -> p (t o)", p=P))
            bias_warm = consts.tile([P, CT], F32, name="bias_warm", tag="bias_warm")
            nc.vector.tensor_copy(bias_warm[:], bias_sb[:])
            bias_warm2 = consts.tile([P, CT], F32, name="bias_warm2", tag="bias_warm2")
            nc.scalar.activation(bias_warm2[:], bias_sb[:],
                                 mybir.ActivationFunctionType.Identity)

            # ---- input activations (pre-transposed on host) ----
            xq_sb = [acts.tile([P, NQ], BD, name=f"xq{i}", tag=f"xq{i}") for i in range(CT)]
            xk_sb = [acts.tile([P, N], BD, name=f"xk{i}", tag=f"xk{i}") for i in range(CT)]
            for i in range(CT):
                nc.sync.dma_start(xq_sb[i][:], xqT[i * P:(i + 1) * P, :])

            qT_sb = [acts.tile([P, NQ], BD, name=f"qT{i}", tag=f"qT{i}") for i in range(CT)]
            kT_sb = [acts.tile([P, N], BD, name=f"kT{i}", tag=f"kT{i}") for i in range(CT)]
            v_sb = [acts.tile([P, H, VW], BD, name=f"v{i}", tag=f"v{i}") for i in range(NKT)]
            xT_sb = [acts.tile([P, NQ], BD, name=f"xT{i}", tag=f"xT{i}") for i in range(CT)]

            # ---- PE warmup: dummy matmuls cover the initial DMA stall and
            # get the HAM activity window hot before real work arrives ----
            dum = consts.tile([1, NQ], BD, name="dum", tag="dum")
            nc.vector.memset(dum[:], 0.0)
            for _ in range(3):
                wps = avp.tile([VW, NQ], F32, name="av", tag="av")
                for r in range(8):
                    nc.tensor.matmul(wps[0:1, :], dum[:, 0:1], dum[:],
                                     start=(r == 0), stop=(r == 7),
                                     skip_group_check=True)

            # ---- fused emission: projection / attention / attn-sum groups
            # interleaved so the PE never idles long enough to re-throttle ----
            wq_t = [wpool.tile([P, C], BD, name=f"wq_t{i}", tag=f"wq{i}") for i in range(CT)]
            for ct in range(CT):
                nc.sync.dma_start(wq_t[ct][:], wqT[ct * P:(ct + 1) * P, :])
            wk_t = [wpool.tile([P, C], BD, name=f"wk_t{i}", tag=f"wk{i}") for i in range(CT)]
            for ct in range(CT):
                nc.sync.dma_start(wk_t[ct][:], wkvT[ct * P:(ct + 1) * P, 0:C])
            for i in range(CT):
                nc.sync.dma_start(xk_sb[i][:], xkT[i * P:(i + 1) * P, :])
            wv_t = [wpool.tile([P, C], BD, name=f"wv_t{i}", tag=f"wv{i}") for i in range(CT)]
            for ct in range(CT):
                nc.sync.dma_start(wv_t[ct][:], wkvT[ct * P:(ct + 1) * P, C:2 * C])

            def emit_A(ot):
                ps = ppp.tile([P, NQ], F32, name="pp", tag="pp")
                for ct in range(CT):
                    nc.tensor.matmul(
                        ps[:], wq_t[ct][:, ot * P:(ot + 1) * P], xq_sb[ct][:],
                        start=(ct == 0), stop=(ct == CT - 1))
                nc.vector.tensor_copy(qT_sb[ot][:], ps[:])

            def emit_B1(ot, hf):
                ps = ppp.tile([P, NQ], F32, name="pp", tag="pp")
                for ct in range(CT):
                    nc.tensor.matmul(
                        ps[:], wk_t[ct][:, ot * P:(ot + 1) * P],
                        xk_sb[ct][:, hf * NQ:(hf + 1) * NQ],
                        start=(ct == 0), stop=(ct == CT - 1))
                nc.vector.tensor_copy(kT_sb[ot][:, hf * NQ:(hf + 1) * NQ], ps[:])

            def emit_B2(nt, hf):
                ps = ppp.tile([P, NQ], F32, name="pp", tag="pp")
                for ct in range(CT):
                    nc.tensor.matmul(
                        ps[:], xk_sb[ct][:, nt * P:(nt + 1) * P],
                        wv_t[ct][:, hf * NQ:(hf + 1) * NQ],
                        start=(ct == 0), stop=(ct == CT - 1))
                nc.vector.tensor_copy(
                    v_sb[nt][:, hf * (H // 2):(hf + 1) * (H // 2), 0:D],
                    ps.rearrange("p (h d) -> p h d", d=D))

            def emit_E(nt, scale_on_act=False):
                ps = ppp.tile([P, NQ], F32, name="pp", tag="pp")
                for ct in range(CT):
                    nc.tensor.matmul(
                        ps[:], kT_sb[ct][:, nt * P:(nt + 1) * P], qT_sb[ct][:],
                        start=(ct == 0), stop=(ct == CT - 1))
                ast = stage.tile([P, NQ], F32, name=f"ast{nt}", tag=f"ast{nt}", bufs=1)
                if scale_on_act:
                    nc.scalar.activation(ast[:], ps[:], Cpy, scale=ATTN_SCALE)
                else:
                    nc.vector.tensor_scalar_mul(ast[:], ps[:], ATTN_SCALE)
                nc.sync.dma_start(attnT[nt * P:(nt + 1) * P, :], ast[:])

            def emit_head(h, inner=None):
                ht, hr = h // 2, (h % 2) * D
                av = avp.tile([VW, NQ], F32, name="av", tag="av")
                for j in range(NKT // SCB):
                    sc = scp.tile([P, SCB, NQ], F32, name="sc", tag="sc")
                    for tt in range(SCB):
                        t = j * SCB + tt
                        nc.tensor.matmul(
                            sc[:, tt, :],
                            kT_sb[ht][hr:hr + D, t * P:(t + 1) * P],
                            qT_sb[ht][hr:hr + D, :],
                            start=True, stop=True)
                    ex = acts.tile([P, SCB, NQ], BD, name=f"ex{j}", tag=f"ex{j}")
                    nc.scalar.activation(ex[:], sc[:], Exp, scale=SCALE)
                    for f in (inner or {}).get(j, []):
                        f()
                    for tt in range(SCB):
                        t = j * SCB + tt
                        nc.tensor.matmul(
                            av[:], v_sb[t][:, h, :], ex[:, tt, :],
                            start=(t == 0), stop=(t == NKT - 1),
                            skip_group_check=True)
                rec = stage.tile([D, NQ], F32, name="rec", tag="rec")
                nc.vector.reciprocal(rec[:, 0:NQ // 2], av[D:VW, 0:NQ // 2])
                nc.vector.reciprocal(rec[:, NQ // 2:], av[D:VW, NQ // 2:])
                nc.vector.tensor_mul(xT_sb[ht][hr:hr + D, :], av[0:D, :], rec[:])

            # prelude: just what head 0's QK needs; v ones columns; the
            # hf=0 V-projection groups run inside head 0 between exp and AV
            for nt in range(NKT):
                nc.vector.memset(v_sb[nt][:, :, D:VW], 1.0)
            emit_A(0)
            emit_B1(0, 0)
            emit_B1(0, 1)

            def emit_head0():
                ht, hr = 0, 0
                av = avp.tile([VW, NQ], F32, name="av", tag="av")
                exs = []
                for j in range(NKT // SCB):
                    sc = scp.tile([P, SCB, NQ], F32, name="sc", tag="sc")
                    for tt in range(SCB):
                        t = j * SCB + tt
                        nc.tensor.matmul(
                            sc[:, tt, :],
                            kT_sb[ht][hr:hr + D, t * P:(t + 1) * P],
                            qT_sb[ht][hr:hr + D, :],
                            start=True, stop=True)
                    ex = acts.tile([P, SCB, NQ], BD, name=f"ex{j}", tag=f"ex{j}")
                    nc.scalar.activation(ex[:], sc[:], Exp, scale=SCALE)
                    exs.append(ex)
                emit_A(1)
                emit_B1(1, 0)
                emit_B1(1, 1)
                wps = avp.tile([VW, NQ], F32, name="av", tag="av")
                for r in range(16):
                    nc.tensor.matmul(wps[0:1, :], dum[:, 0:1], dum[:],
                                     start=(r == 0), stop=(r == 15),
                                     skip_group_check=True)
                for nt in range(NKT):
                    emit_B2(nt, 0)
                for j in range(NKT // SCB):
                    for tt in range(SCB):
                        t = j * SCB + tt
                        nc.tensor.matmul(
                            av[:], v_sb[t][:, 0, :], exs[j][:, tt, :],
                            start=(t == 0), stop=(t == NKT - 1),
                            skip_group_check=True)
                rec = stage.tile([D, NQ], F32, name="rec", tag="rec")
                nc.vector.reciprocal(rec[:, 0:NQ // 2], av[D:VW, 0:NQ // 2])
                nc.vector.reciprocal(rec[:, NQ // 2:], av[D:VW, NQ // 2:])
                nc.vector.tensor_mul(xT_sb[0][0:D, :], av[0:D, :], rec[:])
            fillers = {
                0: [lambda: emit_B2(0, 1), lambda: emit_B2(1, 1)],
                1: [lambda: emit_A(2), lambda: emit_B2(2, 1), lambda: emit_B2(3, 1)],
                2: [lambda: emit_B1(2, 0), lambda: emit_B1(2, 1), lambda: emit_B2(4, 1)],
                3: [lambda: emit_A(3), lambda: emit_B2(5, 1), lambda: emit_B2(6, 1)],
                4: [lambda: emit_B1(3, 0), lambda: emit_B1(3, 1), lambda: emit_B2(7, 1)],
                5: [lambda: emit_A(4), lambda: emit_B1(4, 0)],
                6: [lambda: emit_B1(4, 1), lambda: emit_A(5)],
                7: [lambda: emit_B1(5, 0), lambda: emit_B1(5, 1)],
                8: [lambda: emit_A(6), lambda: emit_B1(6, 0)],
                9: [lambda: emit_B1(6, 1), lambda: emit_A(7)],
                10: [lambda: emit_B1(7, 0), lambda: emit_B1(7, 1)],
                12: [lambda: emit_E(0), lambda: emit_E(1)],
                13: [lambda: emit_E(2), lambda: emit_E(3)],
                14: [lambda: emit_E(4), lambda: emit_E(5)],
                15: [lambda: emit_E(6, True)],
            }
            for h in range(H):
                if h == 0:
                    emit_head0()
                else:
                    emit_head(h)
                for f in fillers.get(h, []):
                    f()

            # ---- stage D: outT[co, nq] = sum_ci wpT[ci, co] xT[ci, nq] + bp ----
            wp_t = [wpool.tile([P, C], BD, name=f"wp_t{i}", tag=f"wq{i}") for i in range(CT)]
            for ct in range(CT):
                nc.sync.dma_start(wp_t[ct][:], wpT[ct * P:(ct + 1) * P, :])
            for ot in range(CT):
                ps = ppp.tile([P, NQ], F32, name="pp", tag="pp")
                for ct in range(CT):
                    nc.tensor.matmul(
                        ps[:], wp_t[ct][:, ot * P:(ot + 1) * P], xT_sb[ct][:],
                        start=(ct == 0), stop=(ct == CT - 1))
                ost = stage.tile([P, NQ], F32, name=f"ost{ot}", tag=f"ost{ot}", bufs=1)
                nc.scalar.activation(ost[:], ps[:], Cpy, bias=bias_sb[:, ot:ot + 1])
                nc.sync.dma_start(outT[ot * P:(ot + 1) * P, :], ost[:])
                if ot == 1:
                    emit_E(7, True)

    _fix_wait_overflow(nc)
    nc.finalize()
    return nc


def _fix_wait_overflow(nc):
    """Walrus's per-instruction ISA structs carry a single sync-wait slot,
    but Tile sometimes attaches two or three.  Three sound repairs:
    - DMA instructions: drop waits on a queue semaphore the instruction
      itself increments (per-engine descriptor FIFO makes them implicit);
    - compute engines: drop own-semaphore waits whose producing instruction
      retired >=3 instructions earlier on the same in-order engine;
    - matmuls: move leftover extra waits onto the immediately-preceding
      Ldweights (no waits, no updates, so no cycle risk)."""
    skip = ("InstDrain", "InstEventSemaphore")
    # Split over-subscribed tail drains: one wait per InstDrain.
    for block in nc.m.functions[0].blocks:
        edits = []
        for idx, inst in enumerate(block.instructions):
            si = getattr(inst, "sync_info", None)
            if (inst.__class__.__name__ == "InstDrain" and si is not None
                    and len(si.on_wait) > 1):
                extra = []
                while len(si.on_wait) > 1:
                    extra.append(si.on_wait.pop())
                pres = []
                for w in extra:
                    d = mybir.InstDrain(
                        name=nc.get_next_instruction_name(),
                        ins=[], outs=[], bass_is_fusable=False)
                    d.engine = inst.engine
                    d.sync_info = mybir.SyncInfo(on_wait=[w], on_update=[])
                    pres.append(d)
                edits.append((idx, pres))
        for idx, pres in reversed(edits):
            for d in reversed(pres):
                block.instructions.insert(idx, d)
    for block in nc.m.functions[0].blocks:
        pos_by_eng = {}
        prev_by_eng = {}
        inc_hist = {}      # (eng, sem) -> [(stream_pos, cum_after)]
        for inst in block.instructions:
            eng = str(getattr(inst, "engine", None))
            pos = pos_by_eng.get(eng, 0)
            si = getattr(inst, "sync_info", None)
            cls = inst.__class__.__name__
            if si is not None and len(si.on_wait) > 1 and cls not in skip:
                ups = {u.ant_name for u in si.on_update}
                keep = []
                for w in si.on_wait:
                    nm = getattr(w, "ant_name", "") or ""
                    if nm in ups and w.wait_value is not None:
                        if cls == "InstDMACopy":
                            continue                      # FIFO-implied
                        hist = inc_hist.get((eng, nm), [])
                        idx = next((p for p, cum in hist
                                    if cum >= w.wait_value), None)
                        if idx is not None and pos - idx - 1 >= 3:
                            continue                      # long retired
                    keep.append(w)
                while len(si.on_wait) > 0:
                    si.on_wait.pop()
                for w in keep:
                    si.on_wait.append(w)
                if len(si.on_wait) > 1:
                    prev = prev_by_eng.get(eng)
                    psi = prev.sync_info if prev is not None else None
                    if psi is None and prev is not None:
                        psi = mybir.SyncInfo(on_wait=[], on_update=[])
                        prev.sync_info = psi
                    if (psi is not None and len(psi.on_wait) == 0
                            and len(psi.on_update) == 0):
                        while len(si.on_wait) > 1:
                            psi.on_wait.append(si.on_wait.pop())
                assert len(si.on_wait) <= 1, (
                    f"{inst.name} ({cls}): still "
                    f"{[(w.ant_name, w.wait_value) for w in si.on_wait]}")
            if si is not None:
                for u in si.on_update:
                    key = (eng, u.ant_name)
                    hist = inc_hist.setdefault(key, [])
                    cum = hist[-1][1] if hist else 0
                    hist.append((pos, cum + (u.update_value or 0)))
            prev_by_eng[eng] = inst
            pos_by_eng[eng] = pos + 1


def make_in_maps(xq, xk, Wq, Wkv, Wp, bp):
    bf16 = mybir.dt.np(BD)
    wqT = np.ascontiguousarray(Wq.T).astype(bf16)
    wkvT = np.ascontiguousarray(Wkv.T).astype(bf16)
    wpT = np.ascontiguousarray(Wp.T).astype(bf16)
    bpc = np.ascontiguousarray(bp.reshape(C, 1))
    in_maps = []
    for c in range(8):
        b, qh = c // 2, c % 2
        in_maps.append({
            "xqT": np.ascontiguousarray(xq[b, qh * NQ:(qh + 1) * NQ, :].T).astype(bf16),
            "xkT": np.ascontiguousarray(xk[b].T).astype(bf16),
            "wqT": wqT, "wkvT": wkvT, "wpT": wpT, "bp": bpc,
        })
    return in_maps


def gather(results):
    out_full = np.empty((N, B, C), np.float32)
    attn_full = np.empty((B, N, N), np.float32)
    for c in range(8):
        b, qh = c // 2, c % 2
        out_full[qh * NQ:(qh + 1) * NQ, b, :] = results[c]["outT"].T
        attn_full[b, qh * NQ:(qh + 1) * NQ, :] = results[c]["attnT"].T
    return out_full, attn_full


def kernel(xq, xk, xv, Wq, Wkv, Wp, bp):
    from concourse.bass_utils import run_bass_kernel_spmd

    if "nc" not in _CACHE:
        _CACHE["nc"] = build_nc()
    nc = _CACHE["nc"]
    in_maps = make_in_maps(
        np.asarray(xq, np.float32), np.asarray(xk, np.float32),
        np.asarray(Wq, np.float32), np.asarray(Wkv, np.float32),
        np.asarray(Wp, np.float32), np.asarray(bp, np.float32),
    )
    res = run_bass_kernel_spmd(nc, in_maps, core_ids=list(range(8)))
    return gather(res.results)
